# revision 22
# baseline (speedup 1.0000x reference)
"""Bahdanau-style attention kernel for Trainium2, 8 NeuronCores.

Reference computation (per batch b):
    score  = tanh(c @ W1 + W1_b + (h @ W2 + W2_b)[None, :])   # [T, U]
    logits = score @ V_w (+ V_b, cancels in softmax)          # [T, 1]
    attn   = softmax(logits over T)
    out    = sum_t attn[t] * c[t, :]                          # [D]

Sharding: pure data-parallel over batch B=64 across 8 cores (8 batches/core).
No collectives; host concatenates per-core outputs.

Host-side marshalling: c is cast to bf16 and shipped transposed [BL, D, T]
(the only layout the kernel needs). All FLOPs run on device.

Per-core dataflow ([u,t] orientation), per batch (T=2048 = 4 t-supers of 512):
  - one 2MB DMA for cT [d128, (k t)]
  - main matmul on TensorE: psum_uc[u128, t512] += W1_chunk.T @ cT_chunk (bf16)
  - tanh on ScalarE with per-partition bias = (h@W2 + b)[u-chunk] -> score^T bf16
  - V-dot split off TensorE: per-partition V-scale on DVE (tensor_scalar 4x
    mode) + fused scale-add pair tree split DVE/GpSimd -> s4[u128, t512] where
    sum_p s4[p,t] = logits[t]; one ones-matmul on TensorE does the
    partition-sum AND broadcasts logits to all 128 partitions.
  - exp on ScalarE -> w_row bf16, accum_out -> per-super softmax denominator
  - pass-2: fused multiply-reduce (scalar_tensor_tensor accum_out) per
    (d-chunk, super), split DVE / GpSimd:
      ctx[d-chunk, slot] = sum_t cT_chunk * w_bcast
  - per-batch tail: reduce supers, transpose [128,4]->[4,128] on TensorE,
    divide by denominator on DVE, DMA out.
"""

import ml_dtypes
import numpy as np

import concourse.bass as bass
import concourse.tile as tile
from concourse import bacc, bass_isa, mybir
from concourse import bass_utils

B, T, D, U = 64, 2048, 512, 512
NCORES = 8
BL = B // NCORES  # 8 batches per core
KD = D // 128     # 4 contraction chunks
NST = T // 512    # 4 t-supers per batch
F32 = mybir.dt.float32
BF16 = mybir.dt.bfloat16
AF = mybir.ActivationFunctionType
ALU = mybir.AluOpType


def build_nc(n_batch=BL, repeat=1, stage=7, psp_bufs=5, pso_bufs=2, ct_bufs=3,
             score_bufs=12, work_bufs=6, sync_load=True, alloc_mode="stack",
             p2_pool_k=2, use_par=True, p2_wide=False):
    # stage: 1=loads 2=+main-mms 3=+tanh 4=+vdot 5=+exp 6=+pass2 7=full
    nc = bacc.Bacc(None, target_bir_lowering=False)

    ct_ext = nc.declare_dram_parameter("ct", [BL, D, T], BF16, isOutput=False)
    ht_ext = nc.declare_dram_parameter("hT", [128, KD * 16], BF16, isOutput=False)
    w1_ext = nc.declare_dram_parameter("W1_w", [D, U], BF16, isOutput=False)
    b1_ext = nc.declare_dram_parameter("W1_b", [U], F32, isOutput=False)
    w2_ext = nc.declare_dram_parameter("W2_w", [D, U], BF16, isOutput=False)
    b2_ext = nc.declare_dram_parameter("W2_b", [U], F32, isOutput=False)
    v_ext = nc.declare_dram_parameter("V_w", [128, KD], F32, isOutput=False)
    ones_ext = nc.declare_dram_parameter("ones", [128, 128], F32, isOutput=False)
    eye_ext = nc.declare_dram_parameter("eye", [128, 128], F32, isOutput=False)
    out_ext = nc.declare_dram_parameter("out", [BL, D], F32, isOutput=True)

    with tile.TileContext(nc, pool_alloc_mode=alloc_mode) as tc:
        with (
            tc.tile_pool(name="const", bufs=1) as constp,
            tc.tile_pool(name="ct", bufs=ct_bufs) as ctp,
            tc.tile_pool(name="work", bufs=work_bufs) as workp,
            tc.tile_pool(name="score", bufs=score_bufs) as scorep,
            tc.tile_pool(name="sv", bufs=8) as svp,
        ):
            # ---------------- setup (one-time) ----------------
            with tc.tile_pool(name="spsum", bufs=1, space="PSUM") as sps:
                ones_f = constp.tile([128, 128], F32)
                nc.gpsimd.dma_start(ones_f[:], ones_ext[:, :])
                ones_bf = constp.tile([128, 128], BF16)
                nc.scalar.activation(ones_bf[:], ones_f[:], AF.Copy)
                eye_f = constp.tile([128, 128], F32)
                nc.gpsimd.dma_start(eye_f[:], eye_ext[:, :])

                # W1 chunks [d128, (k u)] bf16 (pre-converted on host):
                # lhsT slice [d, u-chunk]
                w1_bf = constp.tile([128, KD * U], BF16)
                nc.gpsimd.dma_start(
                    w1_bf[:].rearrange("p (k u) -> p k u", k=KD),
                    w1_ext.rearrange("(k p) u -> p k u", p=128),
                )
                w2_bf = constp.tile([128, KD * U], BF16)
                nc.scalar.dma_start(
                    w2_bf[:].rearrange("p (k u) -> p k u", k=KD),
                    w2_ext.rearrange("(k p) u -> p k u", p=128),
                )

                # hT [d128, (k 16)] bf16 pre-transposed on host
                hT_bf = constp.tile([128, KD * 16], BF16)
                nc.sync.dma_start(hT_bf[:], ht_ext[:, :])

                b1_f = constp.tile([1, U], F32)
                nc.gpsimd.dma_start(b1_f[:], b1_ext[None, :])
                b2_f = constp.tile([1, U], F32)
                nc.gpsimd.dma_start(b2_f[:], b2_ext[None, :])
                b12_f = constp.tile([1, U], F32)
                nc.vector.tensor_add(b12_f[:], b1_f[:], b2_f[:])
                b12_bf = constp.tile([1, U], BF16)
                nc.scalar.activation(b12_bf[:], b12_f[:], AF.Copy)

                # hb[b, u] = h[b] @ W2 + W1_b + W2_b   (rows 0:BL valid)
                ps_hb = sps.tile([16, U], F32)
                for k in range(KD):
                    nc.tensor.matmul(
                        ps_hb[:],
                        lhsT=hT_bf[:, 16 * k : 16 * (k + 1)],
                        rhs=w2_bf[:, U * k : U * (k + 1)],
                        start=(k == 0),
                        stop=False,
                    )
                nc.tensor.matmul(
                    ps_hb[:], lhsT=ones_bf[0:1, 0:16], rhs=b12_bf[:],
                    start=False, stop=True,
                )
                hbr_f = constp.tile([16, U], F32)
                nc.scalar.activation(hbr_f[:], ps_hb[:], AF.Copy)
                # transpose to hbT [u128, (k 16)] f32 (tanh bias columns)
                ps_hbt = sps.tile([128, KD * 16], F32)
                for k in range(KD):
                    nc.tensor.transpose(
                        ps_hbt[:, 16 * k : 16 * (k + 1)],
                        hbr_f[0:16, 128 * k : 128 * (k + 1)],
                        eye_f[0:16, 0:16],
                    )
                hbT_f = constp.tile([128, KD * 16], F32)
                nc.scalar.activation(hbT_f[:], ps_hbt[:], AF.Copy)

                # V columns: vcol_f[u128, k] = V[u-chunk k] (host pre-arranged)
                vcol_f = constp.tile([128, KD], F32)
                nc.scalar.dma_start(vcol_f[:], v_ext[:, :])

            # ---------------- main loop ----------------
            with (
                tc.tile_pool(name="psum_s", bufs=psp_bufs, space="PSUM") as psp,
                tc.tile_pool(name="psum_o", bufs=pso_bufs, space="PSUM") as psop,
                tc.tile_pool(name="psum_t", bufs=1, space="PSUM") as ptp,
            ):
                for rep in range(repeat):
                  ctxn_all = constp.tile([128, n_batch * KD], F32, tag=f"ctxnall{rep}")
                  for b in range(n_batch):
                    cts = ctp.tile([128, KD * T], BF16)
                    load_eng = nc.sync if sync_load else nc.gpsimd
                    load_eng.dma_start(
                        cts[:].rearrange("p (k t) -> p k t", k=KD),
                        ct_ext[b].rearrange("(k p) t -> p k t", p=128),
                    )
                    if stage < 2:
                        continue
                    if not p2_wide:
                        ctx_all = workp.tile([128, KD * NST], F32, tag="ctxall")
                    else:
                        wb_all = workp.tile([128, T], BF16, tag="wball")
                    s_all = workp.tile([128, NST], F32, tag="sall")
                    for st in range(NST):
                        t0 = 512 * st
                        pss = []
                        for uc in range(KD):
                            ps = psp.tile([128, 512], F32, tag="ps")
                            pss.append(ps)
                            for k in range(KD):
                                nc.tensor.matmul(
                                    ps[:],
                                    lhsT=w1_bf[:, U * k + 128 * uc : U * k + 128 * (uc + 1)],
                                    rhs=cts[:, T * k + t0 : T * k + t0 + 512],
                                    start=(k == 0),
                                    stop=(k == KD - 1),
                                )
                        if stage < 3:
                            continue
                        scs = []
                        for uc in range(KD):
                            score = scorep.tile([128, 512], BF16, tag="score")
                            scs.append(score)
                            nc.scalar.activation(
                                score[:], pss[uc][:], AF.Tanh,
                                bias=hbT_f[:, 16 * uc + b : 16 * uc + b + 1],
                            )
                        if stage < 4:
                            continue
                        # s4[p,t] with sum_p s4[p,t] = logits[t]:
                        # Pool: a0 = s0*V0, a1 = s1*V1, a01 = a0+a1, a3 = s3*V3
                        # DVE:  a23 = s2*V2 + a3 (fused STT), s4 = a01+a23
                        # (TensorScalarPtr is illegal on Pool in walrus codegen)
                        a0 = svp.tile([128, 512], BF16, tag="a0")
                        nc.gpsimd.tensor_scalar_mul(a0[:], scs[0][:], vcol_f[:, 0:1])
                        a1 = svp.tile([128, 512], BF16, tag="a1")
                        nc.gpsimd.tensor_scalar_mul(a1[:], scs[1][:], vcol_f[:, 1:2])
                        a01 = svp.tile([128, 512], BF16, tag="a01")
                        nc.gpsimd.tensor_add(a01[:], a0[:], a1[:])
                        a3 = svp.tile([128, 512], BF16, tag="a3")
                        nc.gpsimd.tensor_scalar_mul(a3[:], scs[3][:], vcol_f[:, 3:4])
                        a23 = svp.tile([128, 512], BF16, tag="a23")
                        nc.vector.scalar_tensor_tensor(
                            out=a23[:], in0=scs[2][:], scalar=vcol_f[:, 2:3],
                            in1=a3[:], op0=ALU.mult, op1=ALU.add,
                        )
                        s4 = svp.tile([128, 512], BF16, tag="s4")
                        nc.vector.tensor_add(s4[:], a01[:], a23[:])
                        # partition-sum + broadcast: lg[p,t] = logits[t]
                        if use_par:
                            lg = svp.tile([128, 512], F32, tag="lg")
                            nc.gpsimd.partition_all_reduce(
                                lg[:], s4[:], channels=128,
                                reduce_op=bass_isa.ReduceOp.add,
                            )
                        else:
                            lg = psop.tile([128, 512], F32, tag="pso")
                            nc.tensor.matmul(
                                lg[:], lhsT=ones_bf[:, :], rhs=s4[:],
                                start=True, stop=True,
                            )
                        if stage < 5:
                            continue
                        wtile = wb_all if p2_wide else workp.tile(
                            [128, 512], BF16, tag="wb"
                        )
                        wslice = wtile[:, t0 : t0 + 512] if p2_wide else wtile[:]
                        nc.scalar.activation(
                            wslice, lg[:], AF.Exp,
                            accum_out=s_all[:, st : st + 1],
                        )
                        if stage < 6:
                            continue
                        if not p2_wide:
                            for k in range(KD):
                                prod2 = workp.tile([128, 512], BF16, tag="prod2")
                                if k < p2_pool_k:
                                    # Pool multiply + DVE reduce
                                    nc.gpsimd.tensor_mul(
                                        prod2[:],
                                        cts[:, T * k + t0 : T * k + t0 + 512],
                                        wslice,
                                    )
                                    nc.vector.reduce_sum(
                                        ctx_all[:, NST * k + st : NST * k + st + 1],
                                        prod2[:],
                                        axis=mybir.AxisListType.X,
                                    )
                                else:
                                    # fused multiply-reduce on DVE
                                    nc.vector.scalar_tensor_tensor(
                                        out=prod2[:],
                                        in0=cts[:, T * k + t0 : T * k + t0 + 512],
                                        scalar=1.0,
                                        in1=wslice,
                                        op0=ALU.mult,
                                        op1=ALU.mult,
                                        accum_out=ctx_all[
                                            :, NST * k + st : NST * k + st + 1
                                        ],
                                    )
                    if p2_wide and stage >= 6:
                        ctxs = workp.tile([128, KD], F32, tag="ctxs")
                        for k in range(KD):
                            prod2 = workp.tile([128, T], BF16, tag="prod2w")
                            nc.vector.scalar_tensor_tensor(
                                out=prod2[:],
                                in0=cts[:, T * k : T * (k + 1)],
                                scalar=1.0,
                                in1=wb_all[:],
                                op0=ALU.mult,
                                op1=ALU.mult,
                                accum_out=ctxs[:, k : k + 1],
                            )
                    if stage < 7:
                        continue
                    # ---- per-batch tail (DVE only; transpose batched at end) ----
                    stot = workp.tile([128, 1], F32, tag="stot")
                    nc.vector.reduce_sum(stot[:], s_all[:], axis=mybir.AxisListType.X)
                    invc = workp.tile([128, 1], F32, tag="invc")
                    nc.vector.reciprocal(invc[:], stot[:])
                    if not p2_wide:
                        ctxs = workp.tile([128, KD], F32, tag="ctxs")
                        for k in range(KD):
                            nc.vector.reduce_sum(
                                ctxs[:, k : k + 1],
                                ctx_all[:, NST * k : NST * (k + 1)],
                                axis=mybir.AxisListType.X,
                            )
                    nc.vector.tensor_scalar_mul(
                        ctxn_all[:, KD * b : KD * (b + 1)], ctxs[:], invc[:, 0:1]
                    )
                  if stage >= 7:
                    # ---- end-of-repeat tail: one transpose, one copy, one DMA ----
                    pst = ptp.tile([n_batch * KD, 128], F32, tag="pst")
                    nc.tensor.transpose(pst[:], ctxn_all[:], eye_f[:, :])
                    orows = workp.tile([n_batch * KD, 128], F32, tag="orows")
                    nc.scalar.activation(orows[:], pst[:], AF.Copy)
                    nc.gpsimd.dma_start(
                        out_ext.rearrange("b (k f) -> (b k) f", k=KD), orows[:]
                    )
    nc.compile()
    return nc


_NC_CACHE = None


def _get_nc():
    global _NC_CACHE
    if _NC_CACHE is None:
        _NC_CACHE = build_nc()
    return _NC_CACHE


def make_in_maps(c, h, W1_w, W1_b, W2_w, W2_b, V_w):
    c = np.asarray(c, np.float32)
    cb = c.astype(ml_dtypes.bfloat16)                    # [B, T, D] bf16
    ct = np.ascontiguousarray(cb.swapaxes(1, 2))         # [B, D, T] bf16
    shared = {
        "W1_w": np.ascontiguousarray(
            np.asarray(W1_w, np.float32).astype(ml_dtypes.bfloat16)
        ),
        "W1_b": np.ascontiguousarray(np.asarray(W1_b, np.float32)),
        "W2_w": np.ascontiguousarray(
            np.asarray(W2_w, np.float32).astype(ml_dtypes.bfloat16)
        ),
        "W2_b": np.ascontiguousarray(np.asarray(W2_b, np.float32)),
        # V columns: [p, k] = V[k*128 + p]
        "V_w": np.ascontiguousarray(
            np.asarray(V_w, np.float32).reshape(KD, 128).T
        ),
        "ones": np.ones((128, 128), np.float32),
        "eye": np.eye(128, dtype=np.float32),
    }
    h = np.asarray(h, np.float32)
    in_maps = []
    for i in range(NCORES):
        m = dict(shared)
        m["ct"] = ct[i * BL : (i + 1) * BL]
        # hT[p, k*16 + b] = h[b, k*128 + p], bf16
        hc = h[i * BL : (i + 1) * BL]                       # [BL, D]
        ht = np.zeros((128, KD * 16), np.float32)
        ht[:, :] = np.concatenate(
            [
                np.pad(hc[:, k * 128 : (k + 1) * 128].T, ((0, 0), (0, 16 - BL)))
                for k in range(KD)
            ],
            axis=1,
        )
        m["hT"] = np.ascontiguousarray(ht.astype(ml_dtypes.bfloat16))
        in_maps.append(m)
    return in_maps


def kernel(**inputs):
    in_maps = make_in_maps(
        inputs["c"], inputs["h"], inputs["W1_w"], inputs["W1_b"],
        inputs["W2_w"], inputs["W2_b"], inputs["V_w"],
    )
    nc = _get_nc()
    res = bass_utils.run_bass_kernel_spmd(nc, in_maps, core_ids=list(range(NCORES)))
    out = np.concatenate([np.asarray(r["out"]) for r in res.results], axis=0)
    return out.astype(np.float32)


if __name__ == "__main__":
    rng = np.random.default_rng(0)
    ins = {
        "c": rng.standard_normal((B, T, D), dtype=np.float32),
        "h": rng.standard_normal((B, D), dtype=np.float32),
        "W1_w": rng.standard_normal((D, U), dtype=np.float32) / np.sqrt(D),
        "W1_b": np.zeros((U,), np.float32),
        "W2_w": rng.standard_normal((D, U), dtype=np.float32) / np.sqrt(D),
        "W2_b": np.zeros((U,), np.float32),
        "V_w": rng.standard_normal((U, 1), dtype=np.float32) / np.sqrt(U),
        "V_b": np.zeros((1,), np.float32),
    }
    out = kernel(**ins)
    print("out", out.shape, out.dtype, np.abs(out).mean())


# revision 30
# speedup vs baseline: 8.8304x; 8.8304x over previous
"""Bahdanau-style attention kernel for Trainium2, 8 NeuronCores.

Reference computation (per batch b):
    score  = tanh(c @ W1 + W1_b + (h @ W2 + W2_b)[None, :])   # [T, U]
    logits = score @ V_w (+ V_b, cancels in softmax)          # [T, 1]
    attn   = softmax(logits over T)
    out    = sum_t attn[t] * c[t, :]                          # [D]

Sharding: pure data-parallel over batch B=64 across 8 cores (8 batches/core).
No collectives; host concatenates per-core outputs.

Host-side marshalling: c is cast to bf16 and shipped transposed [BL, D, T]
(the only layout the kernel needs). All FLOPs run on device.

Per-core dataflow ([u,t] orientation), per batch (T=2048 = 4 t-supers of 512):
  - 2MB of cT [d128, (k t)] per batch, split across two DMA queues
    (sync + gpsimd), triple-buffered (pass-2 reads cts late).
  - main matmul on TensorE: psum_uc[u128, t512] += W1_chunk.T @ cT_chunk (bf16)
  - tanh on ScalarE with per-partition bias = (h@W2 + b)[u-chunk] -> score^T bf16
  - V-dot split PE/DVE (vdot_dve chunks on DVE): DVE chunks get per-partition
    V-scale (tensor_scalar, 4x mode) + bf16 add-tree; PE chunks use
    replicated-V lhsT matmuls. One shared PSUM accumulation group (vrep mms +
    one ones-matmul) does the u-partition sum AND broadcasts logits to all
    128 partitions. GpSimd is DMA-trigger only: its tensor ops cost 1.2-7.3us
    per [128,512] op on real HW (sw ucode dispatch), 3-18x the cost model.
  - exp on ScalarE -> w_row bf16, accum_out -> per-super softmax denominator
  - pass-2: fused multiply-reduce on DVE (scalar_tensor_tensor accum_out) per
    (d-chunk, super): ctx[d-chunk, slot] = sum_t cT_chunk * w_bcast
  - per-batch tail: reduce supers, transpose [128,4]->[4,128] on TensorE,
    divide by denominator on DVE, DMA out.

Measured: baseline 160.8us/iter; this version ~140us/iter (repeat-slope, HW).
Remaining gap to the 109us bf16 matmul roofline is mostly unmodeled LDWEIGHTS
overhead (stage=2 loads+matmuls alone measure 133.9us).
"""

import ml_dtypes
import numpy as np

import concourse.bass as bass
import concourse.tile as tile
from concourse import bacc, bass_isa, mybir
from concourse import bass_utils

B, T, D, U = 64, 2048, 512, 512
NCORES = 8
BL = B // NCORES  # 8 batches per core
KD = D // 128     # 4 contraction chunks
NST = T // 512    # 4 t-supers per batch
F32 = mybir.dt.float32
BF16 = mybir.dt.bfloat16
AF = mybir.ActivationFunctionType
ALU = mybir.AluOpType


def build_nc(n_batch=BL, repeat=1, stage=7, psp_bufs=6, pso_bufs=1, ct_bufs=3,
             score_bufs=12, work_bufs=6, sync_load=True, alloc_mode="stack",
             p2_pool_k=0, use_par=False, p2_wide=False, vdot_dve=3, p2_stt=True,
             split_load=True):
    # stage: 1=loads 2=+main-mms 3=+tanh 4=+vdot 5=+exp 6=+pass2 7=full
    nc = bacc.Bacc(None, target_bir_lowering=False)

    ct_ext = nc.declare_dram_parameter("ct", [BL, D, T], BF16, isOutput=False)
    ht_ext = nc.declare_dram_parameter("hT", [128, KD * 16], BF16, isOutput=False)
    w1_ext = nc.declare_dram_parameter("W1_w", [D, U], BF16, isOutput=False)
    b1_ext = nc.declare_dram_parameter("W1_b", [U], F32, isOutput=False)
    w2_ext = nc.declare_dram_parameter("W2_w", [D, U], BF16, isOutput=False)
    b2_ext = nc.declare_dram_parameter("W2_b", [U], F32, isOutput=False)
    v_ext = nc.declare_dram_parameter("V_w", [128, KD], F32, isOutput=False)
    ones_ext = nc.declare_dram_parameter("ones", [128, 128], F32, isOutput=False)
    eye_ext = nc.declare_dram_parameter("eye", [128, 128], F32, isOutput=False)
    out_ext = nc.declare_dram_parameter("out", [BL, D], F32, isOutput=True)

    with tile.TileContext(nc, pool_alloc_mode=alloc_mode) as tc:
        with (
            tc.tile_pool(name="const", bufs=1) as constp,
            tc.tile_pool(name="ct", bufs=ct_bufs) as ctp,
            tc.tile_pool(name="work", bufs=work_bufs) as workp,
            tc.tile_pool(name="score", bufs=score_bufs) as scorep,
            tc.tile_pool(name="sv", bufs=8) as svp,
        ):
            # ---------------- setup (one-time) ----------------
            with tc.tile_pool(name="spsum", bufs=1, space="PSUM") as sps:
                ones_f = constp.tile([128, 128], F32)
                nc.gpsimd.dma_start(ones_f[:], ones_ext[:, :])
                ones_bf = constp.tile([128, 128], BF16)
                nc.scalar.activation(ones_bf[:], ones_f[:], AF.Copy)
                eye_f = constp.tile([128, 128], F32)
                nc.gpsimd.dma_start(eye_f[:], eye_ext[:, :])

                # W1 chunks [d128, (k u)] bf16 (pre-converted on host):
                # lhsT slice [d, u-chunk]
                w1_bf = constp.tile([128, KD * U], BF16)
                nc.gpsimd.dma_start(
                    w1_bf[:].rearrange("p (k u) -> p k u", k=KD),
                    w1_ext.rearrange("(k p) u -> p k u", p=128),
                )
                w2_bf = constp.tile([128, KD * U], BF16)
                nc.scalar.dma_start(
                    w2_bf[:].rearrange("p (k u) -> p k u", k=KD),
                    w2_ext.rearrange("(k p) u -> p k u", p=128),
                )

                # hT [d128, (k 16)] bf16 pre-transposed on host
                hT_bf = constp.tile([128, KD * 16], BF16)
                nc.sync.dma_start(hT_bf[:], ht_ext[:, :])

                b1_f = constp.tile([1, U], F32)
                nc.gpsimd.dma_start(b1_f[:], b1_ext[None, :])
                b2_f = constp.tile([1, U], F32)
                nc.gpsimd.dma_start(b2_f[:], b2_ext[None, :])
                b12_f = constp.tile([1, U], F32)
                nc.vector.tensor_add(b12_f[:], b1_f[:], b2_f[:])
                b12_bf = constp.tile([1, U], BF16)
                nc.scalar.activation(b12_bf[:], b12_f[:], AF.Copy)

                # hb[b, u] = h[b] @ W2 + W1_b + W2_b   (rows 0:BL valid)
                ps_hb = sps.tile([16, U], F32)
                for k in range(KD):
                    nc.tensor.matmul(
                        ps_hb[:],
                        lhsT=hT_bf[:, 16 * k : 16 * (k + 1)],
                        rhs=w2_bf[:, U * k : U * (k + 1)],
                        start=(k == 0),
                        stop=False,
                    )
                nc.tensor.matmul(
                    ps_hb[:], lhsT=ones_bf[0:1, 0:16], rhs=b12_bf[:],
                    start=False, stop=True,
                )
                hbr_f = constp.tile([16, U], F32)
                nc.scalar.activation(hbr_f[:], ps_hb[:], AF.Copy)
                # transpose to hbT [u128, (k 16)] f32 (tanh bias columns)
                ps_hbt = sps.tile([128, KD * 16], F32)
                for k in range(KD):
                    nc.tensor.transpose(
                        ps_hbt[:, 16 * k : 16 * (k + 1)],
                        hbr_f[0:16, 128 * k : 128 * (k + 1)],
                        eye_f[0:16, 0:16],
                    )
                hbT_f = constp.tile([128, KD * 16], F32)
                nc.scalar.activation(hbT_f[:], ps_hbt[:], AF.Copy)

                # V columns: vcol_f[u128, k] = V[u-chunk k] (host pre-arranged)
                vcol_f = constp.tile([128, KD], F32)
                nc.scalar.dma_start(vcol_f[:], v_ext[:, :])
                # V replicated: vrep_k [u128, 128] bf16, every column = V[chunk k]
                vrep_bf = constp.tile([128, KD * 128], BF16)
                for k in range(KD):
                    nc.vector.tensor_scalar(
                        out=vrep_bf[:, 128 * k : 128 * (k + 1)],
                        in0=ones_f[:, :],
                        scalar1=0.0,
                        scalar2=vcol_f[:, k : k + 1],
                        op0=ALU.mult,
                        op1=ALU.add,
                    )

            # ---------------- main loop ----------------
            with (
                tc.tile_pool(name="psum_s", bufs=psp_bufs, space="PSUM") as psp,
                tc.tile_pool(name="psum_o", bufs=pso_bufs, space="PSUM") as psop,
                tc.tile_pool(name="psum_t", bufs=1, space="PSUM") as ptp,
            ):
                for rep in range(repeat):
                  ctxn_all = constp.tile([128, n_batch * KD], F32, tag=f"ctxnall{rep}")
                  for b in range(n_batch):
                    cts = ctp.tile([128, KD * T], BF16)
                    if split_load:
                        # two DMA queues (sync + gpsimd) halve per-batch load
                        # latency and double aggregate load bandwidth
                        ctv = cts[:].rearrange("p (k t) -> p k t", k=KD)
                        csrc = ct_ext[b].rearrange("(k p) t -> p k t", p=128)
                        nc.sync.dma_start(ctv[:, 0:2], csrc[:, 0:2])
                        nc.gpsimd.dma_start(ctv[:, 2:4], csrc[:, 2:4])
                    else:
                        load_eng = nc.sync if sync_load else nc.gpsimd
                        load_eng.dma_start(
                            cts[:].rearrange("p (k t) -> p k t", k=KD),
                            ct_ext[b].rearrange("(k p) t -> p k t", p=128),
                        )
                    if stage < 2:
                        continue
                    if not p2_wide:
                        ctx_all = workp.tile([128, KD * NST], F32, tag="ctxall")
                    else:
                        wb_all = workp.tile([128, T], BF16, tag="wball")
                    s_all = workp.tile([128, NST], F32, tag="sall")
                    for st in range(NST):
                        t0 = 512 * st
                        pss = []
                        for uc in range(KD):
                            ps = psp.tile([128, 512], F32, tag="ps")
                            pss.append(ps)
                            for k in range(KD):
                                nc.tensor.matmul(
                                    ps[:],
                                    lhsT=w1_bf[:, U * k + 128 * uc : U * k + 128 * (uc + 1)],
                                    rhs=cts[:, T * k + t0 : T * k + t0 + 512],
                                    start=(k == 0),
                                    stop=(k == KD - 1),
                                )
                        if stage < 3:
                            continue
                        scs = []
                        for uc in range(KD):
                            score = scorep.tile([128, 512], BF16, tag="score")
                            scs.append(score)
                            nc.scalar.activation(
                                score[:], pss[uc][:], AF.Tanh,
                                bias=hbT_f[:, 16 * uc + b : 16 * uc + b + 1],
                            )
                        if stage < 4:
                            continue
                        # V-dot, split PE/DVE (vdot_dve chunks on DVE):
                        # chunks [0, nv) via replicated-V matmuls (V applied
                        # inside the matmul); chunks [nv, KD) V-scaled on DVE
                        # (tensor_scalar 4x mode) + add-tree, then one
                        # ones-matmul sums partitions + broadcasts. All
                        # matmuls share one PSUM accumulation group:
                        # lg[p,t] = logits[t] for every p.
                        nv = KD - vdot_dve
                        if vdot_dve > 0:
                            scaled = []
                            for j in range(vdot_dve):
                                uc = nv + j
                                aj = svp.tile([128, 512], BF16, tag=f"a{j}")
                                nc.vector.tensor_scalar_mul(
                                    aj[:], scs[uc][:], vcol_f[:, uc : uc + 1]
                                )
                                scaled.append(aj)
                            lvl = 0
                            while len(scaled) > 1:
                                nxt = []
                                for i2 in range(0, len(scaled) - 1, 2):
                                    s = svp.tile(
                                        [128, 512], BF16, tag=f"s{lvl}_{i2}"
                                    )
                                    nc.vector.tensor_add(
                                        s[:], scaled[i2][:], scaled[i2 + 1][:]
                                    )
                                    nxt.append(s)
                                if len(scaled) % 2:
                                    nxt.append(scaled[-1])
                                scaled = nxt
                                lvl += 1
                        lg = psop.tile([128, 512], F32, tag="pso")
                        nmm = nv + (1 if vdot_dve else 0)
                        im = 0
                        for uc in range(nv):
                            nc.tensor.matmul(
                                lg[:],
                                lhsT=vrep_bf[:, 128 * uc : 128 * (uc + 1)],
                                rhs=scs[uc][:],
                                start=(im == 0),
                                stop=(im == nmm - 1),
                            )
                            im += 1
                        if vdot_dve:
                            nc.tensor.matmul(
                                lg[:], lhsT=ones_bf[:, :], rhs=scaled[0][:],
                                start=(im == 0), stop=(im == nmm - 1),
                            )
                        if stage < 5:
                            continue
                        wtile = wb_all if p2_wide else workp.tile(
                            [128, 512], BF16, tag="wb"
                        )
                        wslice = wtile[:, t0 : t0 + 512] if p2_wide else wtile[:]
                        nc.scalar.activation(
                            wslice, lg[:], AF.Exp,
                            accum_out=s_all[:, st : st + 1],
                        )
                        if stage < 6:
                            continue
                        if not p2_wide:
                            for k in range(KD):
                                prod2 = workp.tile([128, 512], BF16, tag="prod2")
                                if k < p2_pool_k:
                                    # Pool multiply + DVE reduce
                                    nc.gpsimd.tensor_mul(
                                        prod2[:],
                                        cts[:, T * k + t0 : T * k + t0 + 512],
                                        wslice,
                                    )
                                    nc.vector.reduce_sum(
                                        ctx_all[:, NST * k + st : NST * k + st + 1],
                                        prod2[:],
                                        axis=mybir.AxisListType.X,
                                    )
                                elif p2_stt:
                                    # fused multiply-reduce on DVE
                                    nc.vector.scalar_tensor_tensor(
                                        out=prod2[:],
                                        in0=cts[:, T * k + t0 : T * k + t0 + 512],
                                        scalar=1.0,
                                        in1=wslice,
                                        op0=ALU.mult,
                                        op1=ALU.mult,
                                        accum_out=ctx_all[
                                            :, NST * k + st : NST * k + st + 1
                                        ],
                                    )
                                else:
                                    # DVE multiply + DVE reduce
                                    nc.vector.tensor_mul(
                                        prod2[:],
                                        cts[:, T * k + t0 : T * k + t0 + 512],
                                        wslice,
                                    )
                                    nc.vector.reduce_sum(
                                        ctx_all[:, NST * k + st : NST * k + st + 1],
                                        prod2[:],
                                        axis=mybir.AxisListType.X,
                                    )
                    if p2_wide and stage >= 6:
                        ctxs = workp.tile([128, KD], F32, tag="ctxs")
                        for k in range(KD):
                            prod2 = workp.tile([128, T], BF16, tag="prod2w")
                            nc.vector.scalar_tensor_tensor(
                                out=prod2[:],
                                in0=cts[:, T * k : T * (k + 1)],
                                scalar=1.0,
                                in1=wb_all[:],
                                op0=ALU.mult,
                                op1=ALU.mult,
                                accum_out=ctxs[:, k : k + 1],
                            )
                    if stage < 7:
                        continue
                    # ---- per-batch tail (DVE only; transpose batched at end) ----
                    stot = workp.tile([128, 1], F32, tag="stot")
                    nc.vector.reduce_sum(stot[:], s_all[:], axis=mybir.AxisListType.X)
                    invc = workp.tile([128, 1], F32, tag="invc")
                    nc.vector.reciprocal(invc[:], stot[:])
                    if not p2_wide:
                        ctxs = workp.tile([128, KD], F32, tag="ctxs")
                        for k in range(KD):
                            nc.vector.reduce_sum(
                                ctxs[:, k : k + 1],
                                ctx_all[:, NST * k : NST * (k + 1)],
                                axis=mybir.AxisListType.X,
                            )
                    nc.vector.tensor_scalar_mul(
                        ctxn_all[:, KD * b : KD * (b + 1)], ctxs[:], invc[:, 0:1]
                    )
                  if stage >= 7:
                    # ---- end-of-repeat tail: one transpose, one copy, one DMA ----
                    pst = ptp.tile([n_batch * KD, 128], F32, tag="pst")
                    nc.tensor.transpose(pst[:], ctxn_all[:], eye_f[:, :])
                    orows = workp.tile([n_batch * KD, 128], F32, tag="orows")
                    nc.scalar.activation(orows[:], pst[:], AF.Copy)
                    nc.gpsimd.dma_start(
                        out_ext.rearrange("b (k f) -> (b k) f", k=KD), orows[:]
                    )
    nc.compile()
    return nc


_NC_CACHE = None


def _get_nc():
    global _NC_CACHE
    if _NC_CACHE is None:
        _NC_CACHE = build_nc()
    return _NC_CACHE


def make_in_maps(c, h, W1_w, W1_b, W2_w, W2_b, V_w):
    c = np.asarray(c, np.float32)
    cb = c.astype(ml_dtypes.bfloat16)                    # [B, T, D] bf16
    ct = np.ascontiguousarray(cb.swapaxes(1, 2))         # [B, D, T] bf16
    shared = {
        "W1_w": np.ascontiguousarray(
            np.asarray(W1_w, np.float32).astype(ml_dtypes.bfloat16)
        ),
        "W1_b": np.ascontiguousarray(np.asarray(W1_b, np.float32)),
        "W2_w": np.ascontiguousarray(
            np.asarray(W2_w, np.float32).astype(ml_dtypes.bfloat16)
        ),
        "W2_b": np.ascontiguousarray(np.asarray(W2_b, np.float32)),
        # V columns: [p, k] = V[k*128 + p]
        "V_w": np.ascontiguousarray(
            np.asarray(V_w, np.float32).reshape(KD, 128).T
        ),
        "ones": np.ones((128, 128), np.float32),
        "eye": np.eye(128, dtype=np.float32),
    }
    h = np.asarray(h, np.float32)
    in_maps = []
    for i in range(NCORES):
        m = dict(shared)
        m["ct"] = ct[i * BL : (i + 1) * BL]
        # hT[p, k*16 + b] = h[b, k*128 + p], bf16
        hc = h[i * BL : (i + 1) * BL]                       # [BL, D]
        ht = np.zeros((128, KD * 16), np.float32)
        ht[:, :] = np.concatenate(
            [
                np.pad(hc[:, k * 128 : (k + 1) * 128].T, ((0, 0), (0, 16 - BL)))
                for k in range(KD)
            ],
            axis=1,
        )
        m["hT"] = np.ascontiguousarray(ht.astype(ml_dtypes.bfloat16))
        in_maps.append(m)
    return in_maps


def kernel(**inputs):
    in_maps = make_in_maps(
        inputs["c"], inputs["h"], inputs["W1_w"], inputs["W1_b"],
        inputs["W2_w"], inputs["W2_b"], inputs["V_w"],
    )
    nc = _get_nc()
    res = bass_utils.run_bass_kernel_spmd(nc, in_maps, core_ids=list(range(NCORES)))
    out = np.concatenate([np.asarray(r["out"]) for r in res.results], axis=0)
    return out.astype(np.float32)


if __name__ == "__main__":
    rng = np.random.default_rng(0)
    ins = {
        "c": rng.standard_normal((B, T, D), dtype=np.float32),
        "h": rng.standard_normal((B, D), dtype=np.float32),
        "W1_w": rng.standard_normal((D, U), dtype=np.float32) / np.sqrt(D),
        "W1_b": np.zeros((U,), np.float32),
        "W2_w": rng.standard_normal((D, U), dtype=np.float32) / np.sqrt(D),
        "W2_b": np.zeros((U,), np.float32),
        "V_w": rng.standard_normal((U, 1), dtype=np.float32) / np.sqrt(U),
        "V_b": np.zeros((1,), np.float32),
    }
    out = kernel(**ins)
    print("out", out.shape, out.dtype, np.abs(out).mean())
